# revision 29
# baseline (speedup 1.0000x reference)
"""Trainium2 Bass kernel for CausalGNNLayer (per-node-type Linear, MoE-style routing).

Semantics (matching the reference):
    out[n, :] = x[n, :] @ W[node_types[n]].T + b[node_types[n]]
edge_index is unused by the op.

Strategy:
- Host-side routing-aware sharding: stable-sort nodes by type, split each
  type's node list into two halves -> 8 groups (4 types x 2 cores).
- Each core gets its group's rows of x in bf16, packed per-chunk as
  [128 part][4 ktile][chunk_n] so every partition's chunk data is one
  contiguous DRAM run (big DMA descriptors).  bf16 runs the PE at full
  rate while halving HBM traffic vs f32, moving the kernel from
  DMA-bound to PE-bound.
- Flipped matmul orientation: stationary = weight k/o-tile, moving = x,
  PSUM out is [out_ch partition, node free].  Bias is then per-PARTITION,
  so the +bias (and f32->bf16 cast) is a cheap tensor_scalar/activation
  op; it is split across the Vector and Scalar engines so neither stalls
  the PE via PSUM backpressure (8-bank PSUM ring = 2 chunks of runway).
- One batched output store per chunk (Activation HWDGE queue); the w
  load gets the SP queue to itself at the start (first x chunks ride the
  Activation queue) so the first matmul isn't queued behind the x
  prefetch.  First/last chunks are small to shorten pipeline fill/drain.
- Warmup matmuls over a zeroed scratch tile run while w/x load, so the
  PE's HAM clock gate is fully open (2.4 GHz) when real work starts.
- Host upcasts bf16 -> f32 and scatters the 8 shards back into [N, 512].

This does the minimum flops (each node touched by exactly one weight),
unlike the reference's compute-all-4-then-mask.
"""

import numpy as np
import ml_dtypes
from contextlib import ExitStack

import concourse.bass as bass
import concourse.mybir as mybir
import concourse.tile as tile
from concourse.bass_utils import run_bass_kernel_spmd

N_CORES = 8
IN_CH = 512
OUT_CH = 512
NUM_TYPES = 4
P_BLK = 128          # SBUF partition count
KT = IN_CH // P_BLK  # 4 contraction tiles
OT = OUT_CH // P_BLK # 4 output-channel tiles
CHUNK = 512          # steady-state nodes per chunk (PSUM bank free-dim limit)
XBUFS = 3            # x-chunk prefetch depth
N_WARM = 12         # PE warmup matmuls (ramp the HAM clock gate during head)
PSBUFS = 8           # PSUM bank ring depth (4 banks per chunk in flight)
OBUFS = 3            # output staging depth

# Set by test harness to capture HW profile; kernel works without it.
TRACE = False
LAST_RESULTS = None

_compile_cache: dict = {}

_legal_nop_counter = [0]


def _legalize_waits(nc: bass.Bass) -> None:
    """This walrus codegen only encodes ONE sync wait per engine instruction.
    Tile's scheduler attaches several.  Split: hoist all-but-one wait of any
    multi-wait instruction into preceding same-engine NoOps (one wait each) —
    semantically identical (the engine stalls on each wait in program order)."""
    for fn in nc.m.functions:
        for blk in fn.blocks:
            insts = blk.instructions
            out = []
            changed = False
            for inst in insts:
                si = inst.sync_info
                waits = list(si.on_wait) if si is not None and si.on_wait else []
                if len(waits) > 1:
                    changed = True
                    for w in waits[:-1]:
                        _legal_nop_counter[0] += 1
                        nop = mybir.InstNoOp(
                            name=f"waitsplit-{_legal_nop_counter[0]}",
                            ins=[],
                            outs=[],
                            engine=inst.engine,
                        )
                        nop.sync_info = mybir.SyncInfo(on_wait=[w], on_update=[])
                        out.append(nop)
                    inst.sync_info = mybir.SyncInfo(
                        on_wait=[waits[-1]], on_update=list(si.on_update or [])
                    )
                out.append(inst)
            if changed:
                blk.instructions = out


def _chunk_plan(P: int) -> list[int]:
    """Chunk sizes summing to P (all multiples of 128, each <= CHUNK).
    Small chunks at both ends shorten pipeline fill and drain."""
    assert P % P_BLK == 0
    if P <= 1024:
        sizes = []
        left = P
        while left:
            c = min(CHUNK, left)
            sizes.append(c)
            left -= c
        return sizes
    head, tail = [128, 384], [256, 128]
    body = P - sum(head) - sum(tail)
    q, r = divmod(body, CHUNK)
    return head + [CHUNK] * q + ([r] if r else []) + tail


def _build_bass(P: int) -> bass.Bass:
    """One-core program: outT[512, P] = w.T @ xT + bias (same program per core)."""
    nc = bass.Bass("TRN2")
    f32 = mybir.dt.float32
    bf16 = mybir.dt.bfloat16

    sizes = _chunk_plan(P)
    offs = np.cumsum([0] + sizes)

    # x packed per chunk as [128][KT][cn] C-order (one contiguous run per
    # partition per chunk); out packed the same way as [128][OT][cn];
    # w packed as [KT][128][OUT] C-order (= [512,512] row-major).
    xT = nc.dram_tensor("xT", [IN_CH * P], bf16, kind="ExternalInput")
    w = nc.dram_tensor("w", [IN_CH * OUT_CH], bf16, kind="ExternalInput")
    bias = nc.dram_tensor("bias", [P_BLK, OT], f32, kind="ExternalInput")
    out = nc.dram_tensor("out", [OUT_CH * P], bf16, kind="ExternalOutput")

    with ExitStack() as ctx:
        tc = ctx.enter_context(tile.TileContext(nc))
        wp = ctx.enter_context(tc.tile_pool(name="w", bufs=1))
        scp = ctx.enter_context(tc.tile_pool(name="sc", bufs=1))
        bp = ctx.enter_context(tc.tile_pool(name="b", bufs=1))
        xp = ctx.enter_context(tc.tile_pool(name="x", bufs=XBUFS))
        pp = ctx.enter_context(tc.tile_pool(name="ps", bufs=PSBUFS, space="PSUM"))
        op = ctx.enter_context(tc.tile_pool(name="o", bufs=OBUFS))

        # PE warmup: the HAM clock gate needs ~3us of continuous PE work to
        # open to 2.4 GHz.  Burn that time during the head (PE would idle
        # waiting on the w/x loads anyway) on matmuls over a zeroed scratch
        # tile, so the real matmuls start at full clock.
        sc = scp.tile([P_BLK, CHUNK], bf16)
        nc.vector.memset(sc[:], 0)
        for _ in range(N_WARM):
            wps = pp.tile([P_BLK, CHUNK], f32, tag="ps")
            nc.tensor.matmul(
                wps[:], lhsT=sc[:, :P_BLK], rhs=sc[:], start=True, stop=True
            )

        # w rides the SP queue alone so its transfer isn't queued behind the
        # initial x prefetch; the first XBUFS x chunks go out on the
        # Activation queue instead, later chunks move back to SP.
        w_sb = wp.tile([P_BLK, KT, OUT_CH], bf16)
        nc.sync.dma_start(
            w_sb[:], w.ap().rearrange("(k p o) -> p k o", k=KT, p=P_BLK)
        )
        b_sb = bp.tile([P_BLK, OT], f32)
        nc.scalar.dma_start(b_sb[:], bias.ap())

        for c, cn in enumerate(sizes):
            x_sb = xp.tile([P_BLK, KT, CHUNK], bf16, tag="x")
            src = xT.ap()[IN_CH * offs[c] : IN_CH * offs[c + 1]].rearrange(
                "(p k n) -> p k n", p=P_BLK, k=KT
            )
            (nc.scalar if c < XBUFS else nc.sync).dma_start(x_sb[:, :, :cn], src)
            o_sb = op.tile([P_BLK, OT, CHUNK], bf16, tag="o")
            for o in range(OT):
                ps = pp.tile([P_BLK, CHUNK], f32, tag="ps")
                for k in range(KT):
                    nc.tensor.matmul(
                        ps[:, :cn],
                        lhsT=w_sb[:, k, o * P_BLK : (o + 1) * P_BLK],
                        rhs=x_sb[:, k, :cn],
                        start=(k == 0),
                        stop=(k == KT - 1),
                    )
                # bias-add + bf16 cast, split across Vector / Scalar engines
                if o < 2:
                    nc.vector.tensor_scalar_add(
                        o_sb[:, o, :cn], ps[:, :cn], b_sb[:, o : o + 1]
                    )
                else:
                    nc.scalar.activation(
                        o_sb[:, o, :cn],
                        ps[:, :cn],
                        mybir.ActivationFunctionType.Identity,
                        bias=b_sb[:, o : o + 1],
                    )
            dst = out.ap()[OUT_CH * offs[c] : OUT_CH * offs[c + 1]].rearrange(
                "(p o n) -> p o n", p=P_BLK, o=OT
            )
            nc.scalar.dma_start(dst, o_sb[:, :, :cn])
    _legalize_waits(nc)
    return nc


def _get_compiled(P: int) -> bass.Bass:
    if P not in _compile_cache:
        _compile_cache[P] = _build_bass(P)
    return _compile_cache[P]


def _pack_x(xs: np.ndarray, sizes: list[int]) -> np.ndarray:
    """[P, IN_CH] bf16 -> per-chunk [128][KT][cn] C-order, flattened."""
    parts = []
    off = 0
    for cn in sizes:
        blk = xs[off : off + cn]  # [cn, 512]
        off += cn
        # -> [512, cn] -> [KT, 128, cn] -> [128, KT, cn]
        parts.append(blk.T.reshape(KT, P_BLK, cn).transpose(1, 0, 2).reshape(-1))
    return np.concatenate(parts)


def _unpack_out(flat: np.ndarray, sizes: list[int]) -> np.ndarray:
    """Per-chunk [128][OT][cn] C-order -> [P, OUT_CH] f32."""
    P = sum(sizes)
    res = np.empty((P, OUT_CH), np.float32)
    off = 0
    pos = 0
    for cn in sizes:
        blk = flat[pos : pos + OUT_CH * cn].reshape(P_BLK, OT, cn)
        # out channel (o*128 + p) -> [cn, 512]
        res[off : off + cn] = blk.transpose(2, 1, 0).reshape(cn, OUT_CH)
        off += cn
        pos += OUT_CH * cn
    return res


def kernel(x, edge_index, node_types, W, b):
    global LAST_RESULTS
    x = np.asarray(x, dtype=np.float32)
    nt = np.asarray(node_types).astype(np.int64)
    W = np.asarray(W, dtype=np.float32)
    b = np.asarray(b, dtype=np.float32)
    N = x.shape[0]

    # Route nodes: stable sort by type, split each type across 2 cores.
    order = np.argsort(nt, kind="stable")
    counts = np.bincount(nt, minlength=NUM_TYPES)
    groups = []
    start = 0
    for t in range(NUM_TYPES):
        c = int(counts[t])
        idx = order[start : start + c]
        start += c
        h = (c + 1) // 2
        groups.append(idx[:h])
        groups.append(idx[h:])

    P = max(P_BLK, max(len(g) for g in groups))
    P = ((P + P_BLK - 1) // P_BLK) * P_BLK
    sizes = _chunk_plan(P)

    nc = _get_compiled(P)

    bf = ml_dtypes.bfloat16
    x_bf = x.astype(bf)
    in_maps = []
    for gi, g in enumerate(groups):
        t = gi // 2
        xs = np.zeros((P, IN_CH), bf)
        if len(g):
            xs[: len(g)] = x_bf[g]
        in_maps.append(
            {
                "xT": _pack_x(xs, sizes),
                # [KT][128][OUT] C-order: w_flat[k*128+p, o]
                "w": np.ascontiguousarray(W[t].T.astype(bf)).reshape(-1),
                "bias": np.ascontiguousarray(b[t].reshape(OT, P_BLK).T),
            }
        )

    res = run_bass_kernel_spmd(nc, in_maps, list(range(N_CORES)), trace=TRACE)
    LAST_RESULTS = res

    out = np.empty((N, OUT_CH), np.float32)
    for gi, g in enumerate(groups):
        if len(g):
            shard = _unpack_out(np.asarray(res.results[gi]["out"]), sizes)
            out[g] = shard[: len(g)]
    return out
